# revision 36
# baseline (speedup 1.0000x reference)
"""MoE layer (shared expert + top-k routed experts) on 8 trn2 NeuronCores.

Sharding (expert-parallel, per the hint):
  - core e holds routed expert e's weights; tokens are dispatched (gathered)
    to their top-k experts' cores on the host, computed on device, and
    scatter-added back on the host.
  - shared expert + router are replicated; tokens are split 8 ways
    (data-parallel) for the shared-expert FFN and the router softmax
    (the `probs` output).

Device kernel layout trick: activations live transposed ([feature, token])
so both FFN matmuls use natural weight layouts as the stationary operand and
no on-device transposes are needed:
  stage1: A^T[h,tok] = sum_d w1[d,h] * X^T[d,tok]   (lhsT = w1 chunk)
  stage2: Y^T[d,tok] = sum_h wp[h,d] * G^T[h,tok]   (lhsT = wp chunk)
Matmuls run in bf16 (fp32 PSUM accumulation); everything else fp32.
"""

import numpy as np
import ml_dtypes
from contextlib import ExitStack

import concourse.tile as tile
from concourse import bacc, mybir
from concourse.bass_utils import run_bass_kernel_spmd

P = 128
NCORES = 8
NT = 512  # token tile (matmul moving free dim / one PSUM bank of fp32)

TRACE = False  # test.py flips this to get exec_time_ns
LAST_RESULTS = None  # BassKernelResults of the last run (for test.py)

_BF16 = ml_dtypes.bfloat16


def _part_layout(a2d: np.ndarray) -> np.ndarray:
    """[R, N] -> [128, R//128, N] partition-inner-first SBUF layout."""
    R, N = a2d.shape
    assert R % P == 0
    return np.ascontiguousarray(a2d.reshape(R // P, P, N).transpose(1, 0, 2))


def _w1_layout(a2d: np.ndarray) -> np.ndarray:
    """[D, H] -> [128, H//128, D//128, 128] (h-major, for chunked loads).

    lhsT block for (d, h) is [:, h, d, :]: element (k, m) = w[d*128+k, h*128+m].
    """
    Dd, Hh = a2d.shape
    return np.ascontiguousarray(
        a2d.reshape(Dd // P, P, Hh // P, P).transpose(1, 2, 0, 3))


def _unpart_layout(a3d: np.ndarray) -> np.ndarray:
    """[128, RC, N] -> [R, N]."""
    p, rc, n = a3d.shape
    return a3d.transpose(1, 0, 2).reshape(rc * p, n)


def _build(TS: int, C: int, D: int, H: int, E: int):
    """Build the single-core Bass program (same on all 8 cores)."""
    DC = D // P  # 6
    HC = H // P  # 12
    dt = mybir.dt
    af = mybir.ActivationFunctionType
    f32, bf16 = dt.float32, dt.bfloat16

    nc = bacc.Bacc("TRN2", target_bir_lowering=False, debug=False,
                   num_devices=NCORES)

    def din(name, shape, dty=bf16):
        return nc.dram_tensor(name, shape, dty, kind="ExternalInput").ap()

    # Shared-slice tokens (transposed), split into two contiguous halves so
    # the shared FFN can start on half 0 while half 1 is still in flight.
    TS2 = TS // 2
    xt_s = din("xt_s", [P, 2, DC, TS2])
    rwt = din("rwt", [P, DC, E])             # router weights, [D, E] layout
    sw1 = din("sw1", [P, HC, DC, P])         # h-major (see _w1_layout)
    sw2 = din("sw2", [P, HC, DC, P])
    sproj = din("sproj", [P, HC, D])
    xt_g = din("xt_g", [P, DC, C])           # gathered expert tokens
    gate = din("gate", [1, C], f32)          # combine weight per gathered tok
    ew1 = din("ew1", [P, HC, DC, P])
    ew2 = din("ew2", [P, HC, DC, P])
    eproj = din("eproj", [P, HC, D])

    probs_o = nc.dram_tensor("probs_o", [TS, E], f32, kind="ExternalOutput").ap()
    shared_o = nc.dram_tensor("shared_o", [P, DC, TS], f32,
                              kind="ExternalOutput").ap()
    yg_o = nc.dram_tensor("yg_o", [P, DC, C], f32, kind="ExternalOutput").ap()

    with tile.TileContext(nc) as tc, ExitStack() as ctx:
        wpool = ctx.enter_context(tc.tile_pool(name="w", bufs=1))
        gpool = ctx.enter_context(tc.tile_pool(name="g", bufs=2))
        tpool = ctx.enter_context(tc.tile_pool(name="t", bufs=3))
        opool = ctx.enter_context(tc.tile_pool(name="o", bufs=6))
        ppool = ctx.enter_context(tc.tile_pool(name="p", bufs=2, space="PSUM"))
        ppool3 = ctx.enter_context(tc.tile_pool(name="p3", bufs=3, space="PSUM"))
        ppool1 = ctx.enter_context(tc.tile_pool(name="p1", bufs=1, space="PSUM"))

        def load(eng, ap_dram, shape, tag, h_chunks=None, dty=bf16):
            """h_chunks: list of (lo, hi) splits along dim 1 for staged
            arrival (first chunk unblocks compute early)."""
            t = wpool.tile(shape, dty, tag=tag)
            if h_chunks is None:
                eng.dma_start(t[:], ap_dram)
            else:
                for lo, hi in h_chunks:
                    eng.dma_start(t[:, lo:hi], ap_dram[:, lo:hi])
            return t

        # All loads on the sync HWDGE ring: it drains FIFO, so emission
        # order IS arrival priority. Critical path (shared stage-1 on the
        # first token half) first, bulk weights behind.
        xs_sb = wpool.tile([P, 2, DC, TS2], bf16, tag="xs")
        sw1_sb = wpool.tile([P, HC, DC, P], bf16, tag="sw1")
        sw2_sb = wpool.tile([P, HC, DC, P], bf16, tag="sw2")
        rw_sb = wpool.tile([P, DC, E], bf16, tag="rw")
        nc.sync.dma_start(xs_sb[:, 0], xt_s[:, 0])
        nc.sync.dma_start(sw1_sb[:, 0:1], sw1[:, 0:1])
        nc.sync.dma_start(sw2_sb[:, 0:1], sw2[:, 0:1])
        nc.sync.dma_start(xs_sb[:, 1], xt_s[:, 1])
        nc.sync.dma_start(rw_sb[:], rwt)
        # Stage-1 consumes sw1[h] and sw2[h] together: interleave their
        # h-chunks in the FIFO so neither gates the other's matmuls.
        for lo, hi in [(1, 2), (2, 4), (4, 6), (6, 8), (8, 10), (10, 12)]:
            nc.sync.dma_start(sw1_sb[:, lo:hi], sw1[:, lo:hi])
            nc.sync.dma_start(sw2_sb[:, lo:hi], sw2[:, lo:hi])
        sproj_sb = load(nc.sync, sproj, [P, HC, D], "sproj",
                        h_chunks=[(0, 6), (6, HC)])
        xg_sb = load(nc.sync, xt_g, [P, DC, C], "xg")
        ew1_sb = wpool.tile([P, HC, DC, P], bf16, tag="ew1")
        ew2_sb = wpool.tile([P, HC, DC, P], bf16, tag="ew2")
        for lo in range(0, HC, 6):
            nc.sync.dma_start(ew1_sb[:, lo:lo + 6], ew1[:, lo:lo + 6])
            nc.sync.dma_start(ew2_sb[:, lo:lo + 6], ew2[:, lo:lo + 6])
        eproj_sb = load(nc.sync, eproj, [P, HC, D], "eproj",
                        h_chunks=[(0, 6), (6, HC)])
        # Gate: needed only in the expert phase, but its 128x-broadcast read
        # would otherwise run at t=0 on the Pool ring and steal DMA-engine
        # time from the critical loads. Gate it behind a late marker load in
        # the sync FIFO via an explicit scheduler dependency.
        gate_ln = wpool.tile([1, C], f32, tag="gateln")
        marker = nc.sync.dma_start(gate_ln[:], gate[:])
        gate_sb = wpool.tile([P, C], f32, tag="gate")
        bcast = nc.gpsimd.dma_start(gate_sb[:], gate.to_broadcast([P, C]))
        tile.add_dep_helper(bcast.ins, marker.ins,
                            reason="delay gate broadcast past critical loads")

        # ---- PE warm-up while DMAs land: the HAM clock gate releases
        # only after ~3.4us of sustained PE activity, so burn junk
        # matmuls (no data deps) during the otherwise-idle startup.
        warm = wpool.tile([P, NT], bf16, tag="warm")
        nc.vector.memset(warm[:], 0.0)
        for _ in range(10):
            pw = ppool.tile([P, NT], f32, tag="pa")
            nc.tensor.matmul(pw, warm[:, :P], warm[:], start=True, stop=True)

        # ---- SwiGLU FFN (transposed activations), shared or routed ----
        # x_fn(d, t0, n) returns the rhs slice for D-chunk d, tokens
        # [t0, t0+n); tiles lists (t0, n) pairs.
        def ffn(x_fn, w1_sb, w2_sb, wp_sb, tiles, out_dram, gated):
            for t0, n in tiles:
                g_sb = gpool.tile([P, HC, NT], bf16, tag="G")
                for h in range(HC):
                    pa = ppool.tile([P, NT], f32, tag="pa")
                    pb = ppool.tile([P, NT], f32, tag="pb")
                    for d in range(DC):
                        nc.tensor.matmul(pa[:, :n],
                                         w1_sb[:, h, d, :],
                                         x_fn(d, t0, n),
                                         start=(d == 0), stop=(d == DC - 1))
                    for d in range(DC):
                        nc.tensor.matmul(pb[:, :n],
                                         w2_sb[:, h, d, :],
                                         x_fn(d, t0, n),
                                         start=(d == 0), stop=(d == DC - 1))
                    st = tpool.tile([P, NT], f32, tag="silu")
                    nc.scalar.activation(st[:, :n], pa[:, :n], af.Silu)
                    nc.vector.tensor_mul(g_sb[:, h, :n], st[:, :n], pb[:, :n])
                for d in range(DC):
                    py = ppool3.tile([P, NT], f32, tag="py")
                    for h in range(HC):
                        nc.tensor.matmul(py[:, :n],
                                         wp_sb[:, h, d * P:(d + 1) * P],
                                         g_sb[:, h, :n],
                                         start=(h == 0), stop=(h == HC - 1))
                    ot = opool.tile([P, NT], f32, tag="out")
                    if gated:
                        nc.vector.tensor_mul(ot[:, :n], py[:, :n],
                                             gate_sb[:, t0:t0 + n])
                    else:
                        # DVE copy: ~9x faster than an ACT copy for f32 tiles.
                        nc.vector.tensor_copy(ot[:, :n], py[:, :n])
                    # Stores on the scalar HWDGE ring: keeps them out of
                    # the input-load FIFO on the sync ring.
                    nc.scalar.dma_start(out_dram[:, d, t0:t0 + n], ot[:, :n])

        # Shared FFN runs in half-slice tiles: tile 0 depends only on the
        # first xs half, so PE starts ~2us earlier than a full-slice tile.
        def xs_fn(d, t0, n):
            c, off = divmod(t0, TS2)
            assert off + n <= TS2
            return xs_sb[:, c, d, off:off + n]

        def xg_fn(d, t0, n):
            return xg_sb[:, d, t0:t0 + n]

        ffn(xs_fn, sw1_sb, sw2_sb, sproj_sb,
            [(0, TS2), (TS2, TS2)], shared_o, gated=False)

        # ---- Router + softmax (probs output); emitted after the shared
        # FFN so its wait on the second xs half never gates the pipeline.
        for m in range(TS // P):
            c, off = divmod(m * P, TS2)
            pr = ppool1.tile([P, E], f32, tag="pr")
            for d in range(DC):
                nc.tensor.matmul(pr, xs_sb[:, c, d, off:off + P],
                                 rw_sb[:, d, :],
                                 start=(d == 0), stop=(d == DC - 1))
            # No max-subtraction: router logits are O(1) here, exp is safe,
            # and softmax is identical mathematically. (Also avoids a
            # multi-wait Activation that walrus codegen rejects.)
            ex = tpool.tile([P, E], f32, tag="ex")
            nc.scalar.activation(ex, pr, af.Exp)
            sm = tpool.tile([P, 1], f32, tag="sm")
            nc.vector.reduce_sum(sm, ex, axis=mybir.AxisListType.X)
            rs = tpool.tile([P, 1], f32, tag="rs")
            nc.vector.reciprocal(rs, sm)
            pb_sb = tpool.tile([P, E], f32, tag="pbs")
            nc.vector.tensor_scalar_mul(pb_sb, ex, rs)
            nc.scalar.dma_start(probs_o[m * P:(m + 1) * P, :], pb_sb)

        ffn(xg_fn, ew1_sb, ew2_sb, eproj_sb,
            [(t0, min(NT, C - t0)) for t0 in range(0, C, NT)],
            yg_o, gated=True)

    nc.compile()
    return nc


def kernel(x, router_w, sw1, sw2, sproj, ew1, ew2, eproj, top_k):
    global LAST_RESULTS
    x = np.asarray(x, np.float32)
    router_w = np.asarray(router_w, np.float32)
    sw1 = np.asarray(sw1, np.float32)
    sw2 = np.asarray(sw2, np.float32)
    sproj = np.asarray(sproj, np.float32)
    ew1 = np.asarray(ew1, np.float32)
    ew2 = np.asarray(ew2, np.float32)
    eproj = np.asarray(eproj, np.float32)
    K = int(np.asarray(top_k))

    B, T, D = x.shape
    E = router_w.shape[0]
    S, _, H = sw1.shape
    NTOK = B * T
    assert NTOK % NCORES == 0 and E == NCORES
    TS = NTOK // NCORES
    xf = x.reshape(NTOK, D)

    # Host-side routing, for dispatch only (device recomputes probs output).
    logits = xf @ router_w.T
    lmax = logits.max(-1, keepdims=True)
    eexp = np.exp(logits - lmax)
    probs_h = eexp / eexp.sum(-1, keepdims=True)
    top_idx = np.argsort(-probs_h, axis=-1, kind="stable")[:, :K]
    top_val = np.take_along_axis(probs_h, top_idx, -1)
    wts = top_val / top_val.sum(-1, keepdims=True)

    flat_e = top_idx.reshape(-1)
    flat_t = np.repeat(np.arange(NTOK), K)
    flat_w = wts.reshape(-1).astype(np.float32)
    order = np.argsort(flat_e, kind="stable")
    counts = np.bincount(flat_e, minlength=E)
    starts = np.zeros(E + 1, np.int64)
    np.cumsum(counts, out=starts[1:])
    # Capacity rounding is paid in PE time by EVERY core (uniform SPMD
    # shapes), so keep it fine-grained.
    C = int(np.ceil(max(int(counts.max()), 1) / 32) * 32)

    # Shared-expert weights summed over the (size-1 here) shared axis happens
    # naturally: S==1 in this problem; for S>1 fold by summing outputs, which
    # is linear only in sproj — so instead require S==1 or loop.
    assert S == 1, "kernel supports a single shared expert stack"
    sw1_l = _w1_layout(sw1[0]).astype(_BF16)
    sw2_l = _w1_layout(sw2[0]).astype(_BF16)
    sproj_l = _part_layout(sproj[0]).astype(_BF16)
    rwt_l = _part_layout(router_w.T).astype(_BF16)

    tok_lists = []
    in_maps = []
    for e in range(NCORES):
        te = flat_t[order[starts[e]:starts[e + 1]]]
        we = flat_w[order[starts[e]:starts[e + 1]]]
        tok_lists.append(te)
        ncnt = len(te)
        xg = np.zeros((D, C), np.float32)
        xg[:, :ncnt] = xf[te].T
        gate_e = np.zeros((1, C), np.float32)
        gate_e[0, :ncnt] = we
        xs = xf[e * TS:(e + 1) * TS].T  # [D, TS]
        xs_l = _part_layout(xs)  # [128, DC, TS]
        xs2 = np.ascontiguousarray(
            xs_l.reshape(P, D // P, 2, TS // 2).transpose(0, 2, 1, 3))
        in_maps.append({
            "xt_s": xs2.astype(_BF16),
            "rwt": rwt_l,
            "sw1": sw1_l,
            "sw2": sw2_l,
            "sproj": sproj_l,
            "xt_g": _part_layout(xg).astype(_BF16),
            "gate": gate_e,
            "ew1": _w1_layout(ew1[e]).astype(_BF16),
            "ew2": _w1_layout(ew2[e]).astype(_BF16),
            "eproj": _part_layout(eproj[e]).astype(_BF16),
        })

    nc = _build(TS, C, D, H, E)
    res = run_bass_kernel_spmd(
        nc, in_maps, list(range(NCORES)),
        trace=TRACE, trace_cores=list(range(NCORES)) if TRACE else None,
    )
    LAST_RESULTS = res

    out = np.empty((NTOK, D), np.float32)
    probs = np.empty((NTOK, E), np.float32)
    for e in range(NCORES):
        r = res.results[e]
        probs[e * TS:(e + 1) * TS] = r["probs_o"]
        out[e * TS:(e + 1) * TS] = _unpart_layout(r["shared_o"]).T
    for e in range(NCORES):
        te = tok_lists[e]
        yg = _unpart_layout(res.results[e]["yg_o"]).T  # [C, D]
        out[te] += yg[:len(te)]

    return out.reshape(B, T, D), probs.reshape(B, T, E)


# revision 40
# speedup vs baseline: 1.0296x; 1.0296x over previous
"""MoE layer (shared expert + top-k routed experts) on 8 trn2 NeuronCores.

Sharding (expert-parallel, per the hint):
  - core e holds routed expert e's weights; tokens are dispatched (gathered)
    to their top-k experts' cores on the host, computed on device, and
    scatter-added back on the host.
  - shared expert + router are replicated; tokens are split 8 ways
    (data-parallel) for the shared-expert FFN and the router softmax
    (the `probs` output).

Device kernel layout trick: activations live transposed ([feature, token])
so both FFN matmuls use natural weight layouts as the stationary operand and
no on-device transposes are needed:
  stage1: A^T[h,tok] = sum_d w1[d,h] * X^T[d,tok]   (lhsT = w1 chunk)
  stage2: Y^T[d,tok] = sum_h wp[h,d] * G^T[h,tok]   (lhsT = wp chunk)
Matmuls run in bf16 (fp32 PSUM accumulation); everything else fp32.
"""

import numpy as np
import ml_dtypes
from contextlib import ExitStack

import concourse.tile as tile
from concourse import bacc, mybir
from concourse.bass_utils import run_bass_kernel_spmd

P = 128
NCORES = 8
NT = 512  # token tile (matmul moving free dim / one PSUM bank of fp32)

TRACE = False  # test.py flips this to get exec_time_ns
LAST_RESULTS = None  # BassKernelResults of the last run (for test.py)

_BF16 = ml_dtypes.bfloat16


def _part_layout(a2d: np.ndarray) -> np.ndarray:
    """[R, N] -> [128, R//128, N] partition-inner-first SBUF layout."""
    R, N = a2d.shape
    assert R % P == 0
    return np.ascontiguousarray(a2d.reshape(R // P, P, N).transpose(1, 0, 2))


def _w1_layout(a2d: np.ndarray) -> np.ndarray:
    """[D, H] -> [128, H//128, D//128, 128] (h-major, for chunked loads).

    lhsT block for (d, h) is [:, h, d, :]: element (k, m) = w[d*128+k, h*128+m].
    """
    Dd, Hh = a2d.shape
    return np.ascontiguousarray(
        a2d.reshape(Dd // P, P, Hh // P, P).transpose(1, 2, 0, 3))


def _unpart_layout(a3d: np.ndarray) -> np.ndarray:
    """[128, RC, N] -> [R, N]."""
    p, rc, n = a3d.shape
    return a3d.transpose(1, 0, 2).reshape(rc * p, n)


def _build(TS: int, C: int, D: int, H: int, E: int):
    """Build the single-core Bass program (same on all 8 cores)."""
    DC = D // P  # 6
    HC = H // P  # 12
    dt = mybir.dt
    af = mybir.ActivationFunctionType
    f32, bf16 = dt.float32, dt.bfloat16

    nc = bacc.Bacc("TRN2", target_bir_lowering=False, debug=False,
                   num_devices=NCORES)

    def din(name, shape, dty=bf16):
        return nc.dram_tensor(name, shape, dty, kind="ExternalInput").ap()

    # Shared-slice tokens (transposed) with the router weights appended as
    # 8 extra columns — one DMA covers the whole router+shared critical path.
    xt_s = din("xt_s", [P, DC, TS + E])
    sw1 = din("sw1", [P, HC, DC, P])         # h-major (see _w1_layout)
    sw2 = din("sw2", [P, HC, DC, P])
    sproj = din("sproj", [P, HC, D])
    xt_g = din("xt_g", [P, DC, C])           # gathered expert tokens
    gate = din("gate", [1, C], f32)          # combine weight per gathered tok
    ew1 = din("ew1", [P, HC, DC, P])
    ew2 = din("ew2", [P, HC, DC, P])
    eproj = din("eproj", [P, HC, D])

    probs_o = nc.dram_tensor("probs_o", [TS, E], f32, kind="ExternalOutput").ap()
    shared_o = nc.dram_tensor("shared_o", [P, DC, TS], f32,
                              kind="ExternalOutput").ap()
    yg_o = nc.dram_tensor("yg_o", [P, DC, C], f32, kind="ExternalOutput").ap()

    with tile.TileContext(nc) as tc, ExitStack() as ctx:
        wpool = ctx.enter_context(tc.tile_pool(name="w", bufs=1))
        gpool = ctx.enter_context(tc.tile_pool(name="g", bufs=2))
        tpool = ctx.enter_context(tc.tile_pool(name="t", bufs=3))
        opool = ctx.enter_context(tc.tile_pool(name="o", bufs=6))
        ppool = ctx.enter_context(tc.tile_pool(name="p", bufs=2, space="PSUM"))
        ppool3 = ctx.enter_context(tc.tile_pool(name="p3", bufs=3, space="PSUM"))
        ppool1 = ctx.enter_context(tc.tile_pool(name="p1", bufs=1, space="PSUM"))

        def load(eng, ap_dram, shape, tag, h_chunks=None, dty=bf16):
            """h_chunks: list of (lo, hi) splits along dim 1 for staged
            arrival (first chunk unblocks compute early)."""
            t = wpool.tile(shape, dty, tag=tag)
            if h_chunks is None:
                eng.dma_start(t[:], ap_dram)
            else:
                for lo, hi in h_chunks:
                    eng.dma_start(t[:, lo:hi], ap_dram[:, lo:hi])
            return t

        # All loads on the sync HWDGE ring: it drains FIFO, so emission
        # order IS arrival priority. Critical path (router + shared
        # stage-1) first, bulk weights behind.
        xsp_sb = load(nc.sync, xt_s, [P, DC, TS + E], "xs")
        xs_sb = xsp_sb[:, :, :TS]
        rw_sb = xsp_sb[:, :, TS:]
        # Stage-1 consumes sw1[h] and sw2[h] together: interleave their
        # h-chunks in the FIFO so neither gates the other's matmuls.
        sw1_sb = wpool.tile([P, HC, DC, P], bf16, tag="sw1")
        sw2_sb = wpool.tile([P, HC, DC, P], bf16, tag="sw2")
        for lo, hi in [(0, 1), (1, 2), (2, 4), (4, 6), (6, 8), (8, 10),
                       (10, 12)]:
            nc.sync.dma_start(sw1_sb[:, lo:hi], sw1[:, lo:hi])
            nc.sync.dma_start(sw2_sb[:, lo:hi], sw2[:, lo:hi])
        sproj_sb = load(nc.sync, sproj, [P, HC, D], "sproj",
                        h_chunks=[(0, 6), (6, HC)])
        xg_sb = load(nc.sync, xt_g, [P, DC, C], "xg")
        ew1_sb = wpool.tile([P, HC, DC, P], bf16, tag="ew1")
        ew2_sb = wpool.tile([P, HC, DC, P], bf16, tag="ew2")
        for lo in range(0, HC, 6):
            nc.sync.dma_start(ew1_sb[:, lo:lo + 6], ew1[:, lo:lo + 6])
            nc.sync.dma_start(ew2_sb[:, lo:lo + 6], ew2[:, lo:lo + 6])
        eproj_sb = load(nc.sync, eproj, [P, HC, D], "eproj",
                        h_chunks=[(0, 6), (6, HC)])
        # Gate: needed only in the expert phase, but its 128x-broadcast read
        # would otherwise run at t=0 on the Pool ring and steal DMA-engine
        # time from the critical loads. Gate it behind a late marker load in
        # the sync FIFO via an explicit scheduler dependency.
        gate_ln = wpool.tile([1, C], f32, tag="gateln")
        marker = nc.sync.dma_start(gate_ln[:], gate[:])
        gate_sb = wpool.tile([P, C], f32, tag="gate")
        bcast = nc.gpsimd.dma_start(gate_sb[:], gate.to_broadcast([P, C]))
        tile.add_dep_helper(bcast.ins, marker.ins,
                            reason="delay gate broadcast past critical loads")

        # ---- PE warm-up while DMAs land: the HAM clock gate releases
        # only after ~3.4us of sustained PE activity, so burn junk
        # matmuls (no data deps) during the otherwise-idle startup.
        warm = wpool.tile([P, NT], bf16, tag="warm")
        nc.vector.memset(warm[:], 0.0)
        for _ in range(10):
            pw = ppool.tile([P, NT], f32, tag="pa")
            nc.tensor.matmul(pw, warm[:, :P], warm[:], start=True, stop=True)

        # ---- SwiGLU FFN (transposed activations), shared or routed ----
        # x_fn(d, t0, n) returns the rhs slice for D-chunk d, tokens
        # [t0, t0+n); tiles lists (t0, n) pairs.
        def ffn(x_fn, w1_sb, w2_sb, wp_sb, tiles, out_dram, gated):
            for t0, n in tiles:
                g_sb = gpool.tile([P, HC, NT], bf16, tag="G")
                for h in range(HC):
                    pa = ppool.tile([P, NT], f32, tag="pa")
                    pb = ppool.tile([P, NT], f32, tag="pb")
                    for d in range(DC):
                        nc.tensor.matmul(pa[:, :n],
                                         w1_sb[:, h, d, :],
                                         x_fn(d, t0, n),
                                         start=(d == 0), stop=(d == DC - 1))
                    for d in range(DC):
                        nc.tensor.matmul(pb[:, :n],
                                         w2_sb[:, h, d, :],
                                         x_fn(d, t0, n),
                                         start=(d == 0), stop=(d == DC - 1))
                    st = tpool.tile([P, NT], f32, tag="silu")
                    nc.scalar.activation(st[:, :n], pa[:, :n], af.Silu)
                    nc.vector.tensor_mul(g_sb[:, h, :n], st[:, :n], pb[:, :n])
                for d in range(DC):
                    py = ppool3.tile([P, NT], f32, tag="py")
                    for h in range(HC):
                        nc.tensor.matmul(py[:, :n],
                                         wp_sb[:, h, d * P:(d + 1) * P],
                                         g_sb[:, h, :n],
                                         start=(h == 0), stop=(h == HC - 1))
                    ot = opool.tile([P, NT], f32, tag="out")
                    if gated:
                        nc.vector.tensor_mul(ot[:, :n], py[:, :n],
                                             gate_sb[:, t0:t0 + n])
                    else:
                        # DVE copy: ~9x faster than an ACT copy for f32 tiles.
                        nc.vector.tensor_copy(ot[:, :n], py[:, :n])
                    # Stores on the scalar HWDGE ring: keeps them out of
                    # the input-load FIFO on the sync ring.
                    nc.scalar.dma_start(out_dram[:, d, t0:t0 + n], ot[:, :n])

        def xs_fn(d, t0, n):
            return xs_sb[:, d, t0:t0 + n]

        def xg_fn(d, t0, n):
            return xg_sb[:, d, t0:t0 + n]

        ffn(xs_fn, sw1_sb, sw2_sb, sproj_sb,
            [(0, TS)], shared_o, gated=False)

        # ---- Router + softmax (probs output); emitted after the shared
        # FFN stage so its wait on the xs tail never gates the pipeline.
        for m in range(TS // P):
            pr = ppool1.tile([P, E], f32, tag="pr")
            for d in range(DC):
                nc.tensor.matmul(pr, xs_sb[:, d, m * P:(m + 1) * P],
                                 rw_sb[:, d, :],
                                 start=(d == 0), stop=(d == DC - 1))
            # No max-subtraction: router logits are O(1) here, exp is safe,
            # and softmax is identical mathematically. (Also avoids a
            # multi-wait Activation that walrus codegen rejects.)
            ex = tpool.tile([P, E], f32, tag="ex")
            nc.scalar.activation(ex, pr, af.Exp)
            sm = tpool.tile([P, 1], f32, tag="sm")
            nc.vector.reduce_sum(sm, ex, axis=mybir.AxisListType.X)
            rs = tpool.tile([P, 1], f32, tag="rs")
            nc.vector.reciprocal(rs, sm)
            pb_sb = tpool.tile([P, E], f32, tag="pbs")
            nc.vector.tensor_scalar_mul(pb_sb, ex, rs)
            nc.scalar.dma_start(probs_o[m * P:(m + 1) * P, :], pb_sb)

        ffn(xg_fn, ew1_sb, ew2_sb, eproj_sb,
            [(t0, min(NT, C - t0)) for t0 in range(0, C, NT)],
            yg_o, gated=True)

    nc.compile()
    return nc


def kernel(x, router_w, sw1, sw2, sproj, ew1, ew2, eproj, top_k):
    global LAST_RESULTS
    x = np.asarray(x, np.float32)
    router_w = np.asarray(router_w, np.float32)
    sw1 = np.asarray(sw1, np.float32)
    sw2 = np.asarray(sw2, np.float32)
    sproj = np.asarray(sproj, np.float32)
    ew1 = np.asarray(ew1, np.float32)
    ew2 = np.asarray(ew2, np.float32)
    eproj = np.asarray(eproj, np.float32)
    K = int(np.asarray(top_k))

    B, T, D = x.shape
    E = router_w.shape[0]
    S, _, H = sw1.shape
    NTOK = B * T
    assert NTOK % NCORES == 0 and E == NCORES
    TS = NTOK // NCORES
    xf = x.reshape(NTOK, D)

    # Host-side routing, for dispatch only (device recomputes probs output).
    logits = xf @ router_w.T
    lmax = logits.max(-1, keepdims=True)
    eexp = np.exp(logits - lmax)
    probs_h = eexp / eexp.sum(-1, keepdims=True)
    top_idx = np.argsort(-probs_h, axis=-1, kind="stable")[:, :K]
    top_val = np.take_along_axis(probs_h, top_idx, -1)
    wts = top_val / top_val.sum(-1, keepdims=True)

    flat_e = top_idx.reshape(-1)
    flat_t = np.repeat(np.arange(NTOK), K)
    flat_w = wts.reshape(-1).astype(np.float32)
    order = np.argsort(flat_e, kind="stable")
    counts = np.bincount(flat_e, minlength=E)
    starts = np.zeros(E + 1, np.int64)
    np.cumsum(counts, out=starts[1:])
    # Capacity rounding is paid in PE time by EVERY core (uniform SPMD
    # shapes), so keep it fine-grained.
    C = int(np.ceil(max(int(counts.max()), 1) / 32) * 32)

    # Shared-expert weights summed over the (size-1 here) shared axis happens
    # naturally: S==1 in this problem; for S>1 fold by summing outputs, which
    # is linear only in sproj — so instead require S==1 or loop.
    assert S == 1, "kernel supports a single shared expert stack"
    sw1_l = _w1_layout(sw1[0]).astype(_BF16)
    sw2_l = _w1_layout(sw2[0]).astype(_BF16)
    sproj_l = _part_layout(sproj[0]).astype(_BF16)
    rwt_l = _part_layout(router_w.T).astype(_BF16)

    tok_lists = []
    in_maps = []
    for e in range(NCORES):
        te = flat_t[order[starts[e]:starts[e + 1]]]
        we = flat_w[order[starts[e]:starts[e + 1]]]
        tok_lists.append(te)
        ncnt = len(te)
        xg = np.zeros((D, C), np.float32)
        xg[:, :ncnt] = xf[te].T
        gate_e = np.zeros((1, C), np.float32)
        gate_e[0, :ncnt] = we
        xs = xf[e * TS:(e + 1) * TS].T  # [D, TS]
        xsp = np.concatenate([_part_layout(xs), rwt_l], axis=2)
        in_maps.append({
            "xt_s": np.ascontiguousarray(xsp).astype(_BF16),
            "sw1": sw1_l,
            "sw2": sw2_l,
            "sproj": sproj_l,
            "xt_g": _part_layout(xg).astype(_BF16),
            "gate": gate_e,
            "ew1": _w1_layout(ew1[e]).astype(_BF16),
            "ew2": _w1_layout(ew2[e]).astype(_BF16),
            "eproj": _part_layout(eproj[e]).astype(_BF16),
        })

    nc = _build(TS, C, D, H, E)
    res = run_bass_kernel_spmd(
        nc, in_maps, list(range(NCORES)),
        trace=TRACE, trace_cores=list(range(NCORES)) if TRACE else None,
    )
    LAST_RESULTS = res

    out = np.empty((NTOK, D), np.float32)
    probs = np.empty((NTOK, E), np.float32)
    for e in range(NCORES):
        r = res.results[e]
        probs[e * TS:(e + 1) * TS] = r["probs_o"]
        out[e * TS:(e + 1) * TS] = _unpart_layout(r["shared_o"]).T
    for e in range(NCORES):
        te = tok_lists[e]
        yg = _unpart_layout(res.results[e]["yg_o"]).T  # [C, D]
        out[te] += yg[:len(te)]

    return out.reshape(B, T, D), probs.reshape(B, T, E)


# revision 41
# speedup vs baseline: 1.0299x; 1.0003x over previous
"""MoE layer (shared expert + top-k routed experts) on 8 trn2 NeuronCores.

Sharding (expert-parallel, per the hint):
  - core e holds routed expert e's weights; tokens are dispatched (gathered)
    to their top-k experts' cores on the host, computed on device, and
    scatter-added back on the host.
  - shared expert + router are replicated; tokens are split 8 ways
    (data-parallel) for the shared-expert FFN and the router softmax
    (the `probs` output).

Device kernel layout trick: activations live transposed ([feature, token])
so both FFN matmuls use natural weight layouts as the stationary operand and
no on-device transposes are needed:
  stage1: A^T[h,tok] = sum_d w1[d,h] * X^T[d,tok]   (lhsT = w1 chunk)
  stage2: Y^T[d,tok] = sum_h wp[h,d] * G^T[h,tok]   (lhsT = wp chunk)
Matmuls run in bf16 (fp32 PSUM accumulation); everything else fp32.
"""

import numpy as np
import ml_dtypes
from contextlib import ExitStack

import concourse.tile as tile
from concourse import bacc, mybir
from concourse.bass_utils import run_bass_kernel_spmd

P = 128
NCORES = 8
NT = 512  # token tile (matmul moving free dim / one PSUM bank of fp32)

TRACE = False  # test.py flips this to get exec_time_ns
LAST_RESULTS = None  # BassKernelResults of the last run (for test.py)

_BF16 = ml_dtypes.bfloat16


def _part_layout(a2d: np.ndarray) -> np.ndarray:
    """[R, N] -> [128, R//128, N] partition-inner-first SBUF layout."""
    R, N = a2d.shape
    assert R % P == 0
    return np.ascontiguousarray(a2d.reshape(R // P, P, N).transpose(1, 0, 2))


def _w1_layout(a2d: np.ndarray) -> np.ndarray:
    """[D, H] -> [128, H//128, D//128, 128] (h-major, for chunked loads).

    lhsT block for (d, h) is [:, h, d, :]: element (k, m) = w[d*128+k, h*128+m].
    """
    Dd, Hh = a2d.shape
    return np.ascontiguousarray(
        a2d.reshape(Dd // P, P, Hh // P, P).transpose(1, 2, 0, 3))


def _unpart_layout(a3d: np.ndarray) -> np.ndarray:
    """[128, RC, N] -> [R, N]."""
    p, rc, n = a3d.shape
    return a3d.transpose(1, 0, 2).reshape(rc * p, n)


def _build(TS: int, C: int, D: int, H: int, E: int):
    """Build the single-core Bass program (same on all 8 cores)."""
    DC = D // P  # 6
    HC = H // P  # 12
    dt = mybir.dt
    af = mybir.ActivationFunctionType
    f32, bf16 = dt.float32, dt.bfloat16

    nc = bacc.Bacc("TRN2", target_bir_lowering=False, debug=False,
                   num_devices=NCORES)

    def din(name, shape, dty=bf16):
        return nc.dram_tensor(name, shape, dty, kind="ExternalInput").ap()

    # Shared-slice tokens (transposed) with the router weights appended as
    # 8 extra columns — one DMA covers the whole router+shared critical path.
    xt_s = din("xt_s", [P, DC, TS + E])
    sw1 = din("sw1", [P, HC, DC, P])         # h-major (see _w1_layout)
    sw2 = din("sw2", [P, HC, DC, P])
    sproj = din("sproj", [P, HC, D])
    xt_g = din("xt_g", [P, DC, C])           # gathered expert tokens
    gate = din("gate", [1, C], f32)          # combine weight per gathered tok
    ew1 = din("ew1", [P, HC, DC, P])
    ew2 = din("ew2", [P, HC, DC, P])
    eproj = din("eproj", [P, HC, D])

    probs_o = nc.dram_tensor("probs_o", [TS, E], f32, kind="ExternalOutput").ap()
    shared_o = nc.dram_tensor("shared_o", [P, DC, TS], f32,
                              kind="ExternalOutput").ap()
    yg_o = nc.dram_tensor("yg_o", [P, DC, C], f32, kind="ExternalOutput").ap()

    with tile.TileContext(nc) as tc, ExitStack() as ctx:
        wpool = ctx.enter_context(tc.tile_pool(name="w", bufs=1))
        gpool = ctx.enter_context(tc.tile_pool(name="g", bufs=2))
        tpool = ctx.enter_context(tc.tile_pool(name="t", bufs=3))
        opool = ctx.enter_context(tc.tile_pool(name="o", bufs=6))
        ppool = ctx.enter_context(tc.tile_pool(name="p", bufs=2, space="PSUM"))
        ppool3 = ctx.enter_context(tc.tile_pool(name="p3", bufs=3, space="PSUM"))
        ppool1 = ctx.enter_context(tc.tile_pool(name="p1", bufs=1, space="PSUM"))

        def load(eng, ap_dram, shape, tag, h_chunks=None, dty=bf16):
            """h_chunks: list of (lo, hi) splits along dim 1 for staged
            arrival (first chunk unblocks compute early)."""
            t = wpool.tile(shape, dty, tag=tag)
            if h_chunks is None:
                eng.dma_start(t[:], ap_dram)
            else:
                for lo, hi in h_chunks:
                    eng.dma_start(t[:, lo:hi], ap_dram[:, lo:hi])
            return t

        # All loads on the sync HWDGE ring: it drains FIFO, so emission
        # order IS arrival priority. Critical path (router + shared
        # stage-1) first, bulk weights behind.
        xsp_sb = load(nc.sync, xt_s, [P, DC, TS + E], "xs")
        xs_sb = xsp_sb[:, :, :TS]
        rw_sb = xsp_sb[:, :, TS:]
        # Stage-1 consumes sw1[h] and sw2[h] together: interleave their
        # h-chunks in the FIFO so neither gates the other's matmuls.
        sw1_sb = wpool.tile([P, HC, DC, P], bf16, tag="sw1")
        sw2_sb = wpool.tile([P, HC, DC, P], bf16, tag="sw2")
        for lo, hi in [(0, 1), (1, 2), (2, 4), (4, 6), (6, 8), (8, 10),
                       (10, 12)]:
            nc.sync.dma_start(sw1_sb[:, lo:hi], sw1[:, lo:hi])
            nc.sync.dma_start(sw2_sb[:, lo:hi], sw2[:, lo:hi])
        sproj_sb = load(nc.sync, sproj, [P, HC, D], "sproj",
                        h_chunks=[(0, 6), (6, HC)])
        xg_sb = load(nc.sync, xt_g, [P, DC, C], "xg")
        ew1_sb = wpool.tile([P, HC, DC, P], bf16, tag="ew1")
        ew2_sb = wpool.tile([P, HC, DC, P], bf16, tag="ew2")
        for lo in range(0, HC, 6):
            nc.sync.dma_start(ew1_sb[:, lo:lo + 6], ew1[:, lo:lo + 6])
            nc.sync.dma_start(ew2_sb[:, lo:lo + 6], ew2[:, lo:lo + 6])
        eproj_sb = load(nc.sync, eproj, [P, HC, D], "eproj",
                        h_chunks=[(0, 6), (6, HC)])
        # Gate: needed only in the expert phase, but its 128x-broadcast read
        # would otherwise run at t=0 on the Pool ring and steal DMA-engine
        # time from the critical loads. Gate it behind a late marker load in
        # the sync FIFO via an explicit scheduler dependency.
        gate_ln = wpool.tile([1, C], f32, tag="gateln")
        marker = nc.sync.dma_start(gate_ln[:], gate[:])
        gate_sb = wpool.tile([P, C], f32, tag="gate")
        bcast = nc.gpsimd.dma_start(gate_sb[:], gate.to_broadcast([P, C]))
        tile.add_dep_helper(bcast.ins, marker.ins,
                            reason="delay gate broadcast past critical loads")

        # ---- PE warm-up while DMAs land: the HAM clock gate releases
        # only after ~3.4us of sustained PE activity, so burn junk
        # matmuls (no data deps) during the otherwise-idle startup.
        warm = wpool.tile([P, NT], bf16, tag="warm")
        nc.vector.memset(warm[:], 0.0)
        for _ in range(10):
            pw = ppool.tile([P, NT], f32, tag="pa")
            nc.tensor.matmul(pw, warm[:, :P], warm[:], start=True, stop=True)

        # ---- SwiGLU FFN (transposed activations), shared or routed ----
        # x_fn(d, t0, n) returns the rhs slice for D-chunk d, tokens
        # [t0, t0+n); tiles lists (t0, n) pairs.
        def ffn(x_fn, w1_sb, w2_sb, wp_sb, tiles, out_dram, gated):
            for t0, n in tiles:
                g_sb = gpool.tile([P, HC, NT], bf16, tag="G")
                for h in range(HC):
                    pa = ppool.tile([P, NT], f32, tag="pa")
                    pb = ppool.tile([P, NT], f32, tag="pb")
                    for d in range(DC):
                        nc.tensor.matmul(pa[:, :n],
                                         w1_sb[:, h, d, :],
                                         x_fn(d, t0, n),
                                         start=(d == 0), stop=(d == DC - 1))
                    for d in range(DC):
                        nc.tensor.matmul(pb[:, :n],
                                         w2_sb[:, h, d, :],
                                         x_fn(d, t0, n),
                                         start=(d == 0), stop=(d == DC - 1))
                    st = tpool.tile([P, NT], f32, tag="silu")
                    nc.scalar.activation(st[:, :n], pa[:, :n], af.Silu)
                    nc.vector.tensor_mul(g_sb[:, h, :n], st[:, :n], pb[:, :n])
                for d in range(DC):
                    py = ppool3.tile([P, NT], f32, tag="py")
                    for h in range(HC):
                        nc.tensor.matmul(py[:, :n],
                                         wp_sb[:, h, d * P:(d + 1) * P],
                                         g_sb[:, h, :n],
                                         start=(h == 0), stop=(h == HC - 1))
                    ot = opool.tile([P, NT], f32, tag="out")
                    if gated:
                        nc.vector.tensor_mul(ot[:, :n], py[:, :n],
                                             gate_sb[:, t0:t0 + n])
                    else:
                        # DVE copy: ~9x faster than an ACT copy for f32 tiles.
                        nc.vector.tensor_copy(ot[:, :n], py[:, :n])
                    # Stores on the scalar HWDGE ring: keeps them out of
                    # the input-load FIFO on the sync ring.
                    nc.scalar.dma_start(out_dram[:, d, t0:t0 + n], ot[:, :n])

        def xs_fn(d, t0, n):
            return xs_sb[:, d, t0:t0 + n]

        def xg_fn(d, t0, n):
            return xg_sb[:, d, t0:t0 + n]

        ffn(xs_fn, sw1_sb, sw2_sb, sproj_sb,
            [(0, TS)], shared_o, gated=False)

        # ---- Router + softmax (probs output); emitted after the shared
        # FFN stage so its wait on the xs tail never gates the pipeline.
        for m in range(TS // P):
            pr = ppool1.tile([P, E], f32, tag="pr")
            for d in range(DC):
                nc.tensor.matmul(pr, xs_sb[:, d, m * P:(m + 1) * P],
                                 rw_sb[:, d, :],
                                 start=(d == 0), stop=(d == DC - 1))
            # No max-subtraction: router logits are O(1) here, exp is safe,
            # and softmax is identical mathematically. (Also avoids a
            # multi-wait Activation that walrus codegen rejects.)
            ex = tpool.tile([P, E], f32, tag="ex")
            nc.scalar.activation(ex, pr, af.Exp)
            sm = tpool.tile([P, 1], f32, tag="sm")
            nc.vector.reduce_sum(sm, ex, axis=mybir.AxisListType.X)
            rs = tpool.tile([P, 1], f32, tag="rs")
            nc.vector.reciprocal(rs, sm)
            pb_sb = tpool.tile([P, E], f32, tag="pbs")
            nc.vector.tensor_scalar_mul(pb_sb, ex, rs)
            nc.scalar.dma_start(probs_o[m * P:(m + 1) * P, :], pb_sb)

        # Even-split expert tiles: a tiny tail tile (e.g. 64 tokens) cannot
        # hide LDWEIGHTS behind its matmuls, so balance tile sizes instead.
        n_tiles = -(-C // NT)
        e_tiles, rem = [], C
        for i in range(n_tiles):
            s = -(-(rem // (n_tiles - i)) // 32) * 32
            e_tiles.append((C - rem, s))
            rem -= s
        ffn(xg_fn, ew1_sb, ew2_sb, eproj_sb, e_tiles, yg_o, gated=True)

    nc.compile()
    return nc


def kernel(x, router_w, sw1, sw2, sproj, ew1, ew2, eproj, top_k):
    global LAST_RESULTS
    x = np.asarray(x, np.float32)
    router_w = np.asarray(router_w, np.float32)
    sw1 = np.asarray(sw1, np.float32)
    sw2 = np.asarray(sw2, np.float32)
    sproj = np.asarray(sproj, np.float32)
    ew1 = np.asarray(ew1, np.float32)
    ew2 = np.asarray(ew2, np.float32)
    eproj = np.asarray(eproj, np.float32)
    K = int(np.asarray(top_k))

    B, T, D = x.shape
    E = router_w.shape[0]
    S, _, H = sw1.shape
    NTOK = B * T
    assert NTOK % NCORES == 0 and E == NCORES
    TS = NTOK // NCORES
    xf = x.reshape(NTOK, D)

    # Host-side routing, for dispatch only (device recomputes probs output).
    logits = xf @ router_w.T
    lmax = logits.max(-1, keepdims=True)
    eexp = np.exp(logits - lmax)
    probs_h = eexp / eexp.sum(-1, keepdims=True)
    top_idx = np.argsort(-probs_h, axis=-1, kind="stable")[:, :K]
    top_val = np.take_along_axis(probs_h, top_idx, -1)
    wts = top_val / top_val.sum(-1, keepdims=True)

    flat_e = top_idx.reshape(-1)
    flat_t = np.repeat(np.arange(NTOK), K)
    flat_w = wts.reshape(-1).astype(np.float32)
    order = np.argsort(flat_e, kind="stable")
    counts = np.bincount(flat_e, minlength=E)
    starts = np.zeros(E + 1, np.int64)
    np.cumsum(counts, out=starts[1:])
    # Capacity rounding is paid in PE time by EVERY core (uniform SPMD
    # shapes), so keep it fine-grained.
    C = int(np.ceil(max(int(counts.max()), 1) / 32) * 32)

    # Shared-expert weights summed over the (size-1 here) shared axis happens
    # naturally: S==1 in this problem; for S>1 fold by summing outputs, which
    # is linear only in sproj — so instead require S==1 or loop.
    assert S == 1, "kernel supports a single shared expert stack"
    sw1_l = _w1_layout(sw1[0]).astype(_BF16)
    sw2_l = _w1_layout(sw2[0]).astype(_BF16)
    sproj_l = _part_layout(sproj[0]).astype(_BF16)
    rwt_l = _part_layout(router_w.T).astype(_BF16)

    tok_lists = []
    in_maps = []
    for e in range(NCORES):
        te = flat_t[order[starts[e]:starts[e + 1]]]
        we = flat_w[order[starts[e]:starts[e + 1]]]
        tok_lists.append(te)
        ncnt = len(te)
        xg = np.zeros((D, C), np.float32)
        xg[:, :ncnt] = xf[te].T
        gate_e = np.zeros((1, C), np.float32)
        gate_e[0, :ncnt] = we
        xs = xf[e * TS:(e + 1) * TS].T  # [D, TS]
        xsp = np.concatenate([_part_layout(xs), rwt_l], axis=2)
        in_maps.append({
            "xt_s": np.ascontiguousarray(xsp).astype(_BF16),
            "sw1": sw1_l,
            "sw2": sw2_l,
            "sproj": sproj_l,
            "xt_g": _part_layout(xg).astype(_BF16),
            "gate": gate_e,
            "ew1": _w1_layout(ew1[e]).astype(_BF16),
            "ew2": _w1_layout(ew2[e]).astype(_BF16),
            "eproj": _part_layout(eproj[e]).astype(_BF16),
        })

    nc = _build(TS, C, D, H, E)
    res = run_bass_kernel_spmd(
        nc, in_maps, list(range(NCORES)),
        trace=TRACE, trace_cores=list(range(NCORES)) if TRACE else None,
    )
    LAST_RESULTS = res

    out = np.empty((NTOK, D), np.float32)
    probs = np.empty((NTOK, E), np.float32)
    for e in range(NCORES):
        r = res.results[e]
        probs[e * TS:(e + 1) * TS] = r["probs_o"]
        out[e * TS:(e + 1) * TS] = _unpart_layout(r["shared_o"]).T
    for e in range(NCORES):
        te = tok_lists[e]
        yg = _unpart_layout(res.results[e]["yg_o"]).T  # [C, D]
        out[te] += yg[:len(te)]

    return out.reshape(B, T, D), probs.reshape(B, T, E)


# revision 44
# speedup vs baseline: 1.0630x; 1.0321x over previous
"""MoE layer (shared expert + top-k routed experts) on 8 trn2 NeuronCores.

Sharding (expert-parallel, per the hint):
  - core e holds routed expert e's weights; tokens are dispatched (gathered)
    to their top-k experts' cores on the host, computed on device, and
    scatter-added back on the host.
  - shared expert + router are replicated; tokens are split 8 ways
    (data-parallel) for the shared-expert FFN and the router softmax
    (the `probs` output).

Device kernel layout trick: activations live transposed ([feature, token])
so both FFN matmuls use natural weight layouts as the stationary operand and
no on-device transposes are needed:
  stage1: A^T[h,tok] = sum_d w1[d,h] * X^T[d,tok]   (lhsT = w1 chunk)
  stage2: Y^T[d,tok] = sum_h wp[h,d] * G^T[h,tok]   (lhsT = wp chunk)
Matmuls run in bf16 (fp32 PSUM accumulation); everything else fp32.
"""

import numpy as np
import ml_dtypes
from contextlib import ExitStack

import concourse.tile as tile
from concourse import bacc, mybir
from concourse.bass_utils import run_bass_kernel_spmd

P = 128
NCORES = 8
NT = 512  # token tile (matmul moving free dim / one PSUM bank of fp32)

TRACE = False  # test.py flips this to get exec_time_ns
LAST_RESULTS = None  # BassKernelResults of the last run (for test.py)

_BF16 = ml_dtypes.bfloat16


def _part_layout(a2d: np.ndarray) -> np.ndarray:
    """[R, N] -> [128, R//128, N] partition-inner-first SBUF layout."""
    R, N = a2d.shape
    assert R % P == 0
    return np.ascontiguousarray(a2d.reshape(R // P, P, N).transpose(1, 0, 2))


def _w1_layout(a2d: np.ndarray) -> np.ndarray:
    """[D, H] -> [128, H//128, D//128, 128] (h-major, for chunked loads).

    lhsT block for (d, h) is [:, h, d, :]: element (k, m) = w[d*128+k, h*128+m].
    """
    Dd, Hh = a2d.shape
    return np.ascontiguousarray(
        a2d.reshape(Dd // P, P, Hh // P, P).transpose(1, 2, 0, 3))


def _unpart_layout(a3d: np.ndarray) -> np.ndarray:
    """[128, RC, N] -> [R, N]."""
    p, rc, n = a3d.shape
    return a3d.transpose(1, 0, 2).reshape(rc * p, n)


def _build(TS: int, C: int, D: int, H: int, E: int):
    """Build the single-core Bass program (same on all 8 cores)."""
    DC = D // P  # 6
    HC = H // P  # 12
    dt = mybir.dt
    af = mybir.ActivationFunctionType
    f32, bf16 = dt.float32, dt.bfloat16

    nc = bacc.Bacc("TRN2", target_bir_lowering=False, debug=False,
                   num_devices=NCORES)

    def din(name, shape, dty=bf16):
        return nc.dram_tensor(name, shape, dty, kind="ExternalInput").ap()

    # Shared-slice tokens (transposed) with the router weights appended as
    # 8 extra columns — one DMA covers the whole router+shared critical path.
    xt_s = din("xt_s", [P, DC, TS + E])
    sw1 = din("sw1", [P, HC, DC, P])         # h-major (see _w1_layout)
    sw2 = din("sw2", [P, HC, DC, P])
    sproj = din("sproj", [P, HC, D])
    xt_g = din("xt_g", [P, DC, C])           # gathered expert tokens
    gate = din("gate", [1, C], f32)          # combine weight per gathered tok
    ew1 = din("ew1", [P, HC, DC, P])
    ew2 = din("ew2", [P, HC, DC, P])
    eproj = din("eproj", [P, HC, D])

    probs_o = nc.dram_tensor("probs_o", [TS, E], f32, kind="ExternalOutput").ap()
    shared_o = nc.dram_tensor("shared_o", [P, DC, TS], f32,
                              kind="ExternalOutput").ap()
    yg_o = nc.dram_tensor("yg_o", [P, DC, C], f32, kind="ExternalOutput").ap()

    with tile.TileContext(nc) as tc, ExitStack() as ctx:
        wpool = ctx.enter_context(tc.tile_pool(name="w", bufs=1))
        gpool = ctx.enter_context(tc.tile_pool(name="g", bufs=2))
        tpool = ctx.enter_context(tc.tile_pool(name="t", bufs=3))
        opool = ctx.enter_context(tc.tile_pool(name="o", bufs=6))
        ppool = ctx.enter_context(tc.tile_pool(name="p", bufs=2, space="PSUM"))
        ppool3 = ctx.enter_context(tc.tile_pool(name="p3", bufs=3, space="PSUM"))
        ppool1 = ctx.enter_context(tc.tile_pool(name="p1", bufs=1, space="PSUM"))

        def load(eng, ap_dram, shape, tag, h_chunks=None, dty=bf16):
            """h_chunks: list of (lo, hi) splits along dim 1 for staged
            arrival (first chunk unblocks compute early)."""
            t = wpool.tile(shape, dty, tag=tag)
            if h_chunks is None:
                eng.dma_start(t[:], ap_dram)
            else:
                for lo, hi in h_chunks:
                    eng.dma_start(t[:, lo:hi], ap_dram[:, lo:hi])
            return t

        # All loads on the sync HWDGE ring: it drains FIFO, so emission
        # order IS arrival priority. Critical path (router + shared
        # stage-1) first, bulk weights behind.
        xsp_sb = load(nc.sync, xt_s, [P, DC, TS + E], "xs")
        xs_sb = xsp_sb[:, :, :TS]
        rw_sb = xsp_sb[:, :, TS:]
        # Stage-1 consumes sw1[h] and sw2[h] together: interleave their
        # h-chunks in the FIFO so neither gates the other's matmuls.
        sw1_sb = wpool.tile([P, HC, DC, P], bf16, tag="sw1")
        sw2_sb = wpool.tile([P, HC, DC, P], bf16, tag="sw2")
        for lo, hi in [(0, 1), (1, 2), (2, 4), (4, 6), (6, 8), (8, 10),
                       (10, 12)]:
            nc.sync.dma_start(sw1_sb[:, lo:hi], sw1[:, lo:hi])
            nc.sync.dma_start(sw2_sb[:, lo:hi], sw2[:, lo:hi])
        sproj_sb = load(nc.sync, sproj, [P, HC, D], "sproj",
                        h_chunks=[(0, 6), (6, HC)])
        xg_sb = load(nc.sync, xt_g, [P, DC, C], "xg")
        ew1_sb = wpool.tile([P, HC, DC, P], bf16, tag="ew1")
        ew2_sb = wpool.tile([P, HC, DC, P], bf16, tag="ew2")
        for lo in range(0, HC, 6):
            nc.sync.dma_start(ew1_sb[:, lo:lo + 6], ew1[:, lo:lo + 6])
            nc.sync.dma_start(ew2_sb[:, lo:lo + 6], ew2[:, lo:lo + 6])
        eproj_sb = load(nc.sync, eproj, [P, HC, D], "eproj",
                        h_chunks=[(0, 6), (6, HC)])
        # Gate: needed only in the expert phase, but its 128x-broadcast read
        # would otherwise run at t=0 on the Pool ring and steal DMA-engine
        # time from the critical loads. Gate it behind a late marker load in
        # the sync FIFO via an explicit scheduler dependency.
        gate_ln = wpool.tile([1, C], f32, tag="gateln")
        marker = nc.sync.dma_start(gate_ln[:], gate[:])
        gate_sb = wpool.tile([P, C], f32, tag="gate")
        bcast = nc.gpsimd.dma_start(gate_sb[:], gate.to_broadcast([P, C]))
        tile.add_dep_helper(bcast.ins, marker.ins,
                            reason="delay gate broadcast past critical loads")

        # ---- PE warm-up while DMAs land: the HAM clock gate releases
        # only after ~3.4us of sustained PE activity, so burn junk
        # matmuls (no data deps) during the otherwise-idle startup.
        warm = wpool.tile([P, NT], bf16, tag="warm")
        nc.vector.memset(warm[:], 0.0)
        for _ in range(10):
            pw = ppool.tile([P, NT], f32, tag="pa")
            nc.tensor.matmul(pw, warm[:, :P], warm[:], start=True, stop=True)

        # ---- SwiGLU FFN (transposed activations), shared or routed ----
        # x_fn(d, t0, n) returns the rhs slice for D-chunk d, tokens
        # [t0, t0+n); tiles lists (t0, n) pairs.
        def ffn(x_fn, w1_sb, w2_sb, wp_sb, tiles, out_dram, gated):
            for t0, n in tiles:
                g_sb = gpool.tile([P, HC, NT], bf16, tag="G")
                for h in range(HC):
                    pa = ppool.tile([P, NT], f32, tag="pa")
                    pb = ppool.tile([P, NT], f32, tag="pb")
                    for d in range(DC):
                        nc.tensor.matmul(pa[:, :n],
                                         w1_sb[:, h, d, :],
                                         x_fn(d, t0, n),
                                         start=(d == 0), stop=(d == DC - 1))
                    for d in range(DC):
                        nc.tensor.matmul(pb[:, :n],
                                         w2_sb[:, h, d, :],
                                         x_fn(d, t0, n),
                                         start=(d == 0), stop=(d == DC - 1))
                    st = tpool.tile([P, NT], f32, tag="silu")
                    nc.scalar.activation(st[:, :n], pa[:, :n], af.Silu)
                    nc.vector.tensor_mul(g_sb[:, h, :n], st[:, :n], pb[:, :n])
                for d in range(DC):
                    py = ppool3.tile([P, NT], f32, tag="py")
                    for h in range(HC):
                        nc.tensor.matmul(py[:, :n],
                                         wp_sb[:, h, d * P:(d + 1) * P],
                                         g_sb[:, h, :n],
                                         start=(h == 0), stop=(h == HC - 1))
                    ot = opool.tile([P, NT], f32, tag="out")
                    if gated:
                        nc.vector.tensor_mul(ot[:, :n], py[:, :n],
                                             gate_sb[:, t0:t0 + n])
                    else:
                        # DVE copy: ~9x faster than an ACT copy for f32 tiles.
                        nc.vector.tensor_copy(ot[:, :n], py[:, :n])
                    # Alternate stores across both HWDGE rings: halves the
                    # serial ~600ns/enqueue chain in the tail epilogue. The
                    # sync ring's load FIFO is issued (and mostly drained)
                    # before any store reaches it.
                    eng = nc.sync if d % 2 else nc.scalar
                    eng.dma_start(out_dram[:, d, t0:t0 + n], ot[:, :n])

        def xs_fn(d, t0, n):
            return xs_sb[:, d, t0:t0 + n]

        def xg_fn(d, t0, n):
            return xg_sb[:, d, t0:t0 + n]

        ffn(xs_fn, sw1_sb, sw2_sb, sproj_sb,
            [(0, TS)], shared_o, gated=False)

        # ---- Router + softmax (probs output); emitted after the shared
        # FFN stage so its wait on the xs tail never gates the pipeline.
        for m in range(TS // P):
            pr = ppool1.tile([P, E], f32, tag="pr")
            for d in range(DC):
                nc.tensor.matmul(pr, xs_sb[:, d, m * P:(m + 1) * P],
                                 rw_sb[:, d, :],
                                 start=(d == 0), stop=(d == DC - 1))
            # No max-subtraction: router logits are O(1) here, exp is safe,
            # and softmax is identical mathematically. (Also avoids a
            # multi-wait Activation that walrus codegen rejects.)
            ex = tpool.tile([P, E], f32, tag="ex")
            nc.scalar.activation(ex, pr, af.Exp)
            sm = tpool.tile([P, 1], f32, tag="sm")
            nc.vector.reduce_sum(sm, ex, axis=mybir.AxisListType.X)
            rs = tpool.tile([P, 1], f32, tag="rs")
            nc.vector.reciprocal(rs, sm)
            pb_sb = tpool.tile([P, E], f32, tag="pbs")
            nc.vector.tensor_scalar_mul(pb_sb, ex, rs)
            nc.scalar.dma_start(probs_o[m * P:(m + 1) * P, :], pb_sb)

        # Even-split expert tiles: a tiny tail tile (e.g. 64 tokens) cannot
        # hide LDWEIGHTS behind its matmuls, so balance tile sizes instead.
        n_tiles = -(-C // NT)
        e_tiles, rem = [], C
        for i in range(n_tiles):
            if i == n_tiles - 1:
                s = rem
            else:
                s = min(NT, -(-(rem // (n_tiles - i)) // 32) * 32)
            e_tiles.append((C - rem, s))
            rem -= s
        ffn(xg_fn, ew1_sb, ew2_sb, eproj_sb, e_tiles, yg_o, gated=True)

    nc.compile()
    return nc


def kernel(x, router_w, sw1, sw2, sproj, ew1, ew2, eproj, top_k):
    global LAST_RESULTS
    x = np.asarray(x, np.float32)
    router_w = np.asarray(router_w, np.float32)
    sw1 = np.asarray(sw1, np.float32)
    sw2 = np.asarray(sw2, np.float32)
    sproj = np.asarray(sproj, np.float32)
    ew1 = np.asarray(ew1, np.float32)
    ew2 = np.asarray(ew2, np.float32)
    eproj = np.asarray(eproj, np.float32)
    K = int(np.asarray(top_k))

    B, T, D = x.shape
    E = router_w.shape[0]
    S, _, H = sw1.shape
    NTOK = B * T
    assert NTOK % NCORES == 0 and E == NCORES
    TS = NTOK // NCORES
    xf = x.reshape(NTOK, D)

    # Host-side routing, for dispatch only (device recomputes probs output).
    logits = xf @ router_w.T
    lmax = logits.max(-1, keepdims=True)
    eexp = np.exp(logits - lmax)
    probs_h = eexp / eexp.sum(-1, keepdims=True)
    top_idx = np.argsort(-probs_h, axis=-1, kind="stable")[:, :K]
    top_val = np.take_along_axis(probs_h, top_idx, -1)
    wts = top_val / top_val.sum(-1, keepdims=True)

    flat_e = top_idx.reshape(-1)
    flat_t = np.repeat(np.arange(NTOK), K)
    flat_w = wts.reshape(-1).astype(np.float32)
    order = np.argsort(flat_e, kind="stable")
    counts = np.bincount(flat_e, minlength=E)
    starts = np.zeros(E + 1, np.int64)
    np.cumsum(counts, out=starts[1:])
    # Capacity rounding is paid in PE time by EVERY core (uniform SPMD
    # shapes), so keep it fine-grained (8 keeps DMA slices 16B-aligned).
    C = int(np.ceil(max(int(counts.max()), 1) / 8) * 8)

    # Shared-expert weights summed over the (size-1 here) shared axis happens
    # naturally: S==1 in this problem; for S>1 fold by summing outputs, which
    # is linear only in sproj — so instead require S==1 or loop.
    assert S == 1, "kernel supports a single shared expert stack"
    sw1_l = _w1_layout(sw1[0]).astype(_BF16)
    sw2_l = _w1_layout(sw2[0]).astype(_BF16)
    sproj_l = _part_layout(sproj[0]).astype(_BF16)
    rwt_l = _part_layout(router_w.T).astype(_BF16)

    tok_lists = []
    in_maps = []
    for e in range(NCORES):
        te = flat_t[order[starts[e]:starts[e + 1]]]
        we = flat_w[order[starts[e]:starts[e + 1]]]
        tok_lists.append(te)
        ncnt = len(te)
        xg = np.zeros((D, C), np.float32)
        xg[:, :ncnt] = xf[te].T
        gate_e = np.zeros((1, C), np.float32)
        gate_e[0, :ncnt] = we
        xs = xf[e * TS:(e + 1) * TS].T  # [D, TS]
        xsp = np.concatenate([_part_layout(xs), rwt_l], axis=2)
        in_maps.append({
            "xt_s": np.ascontiguousarray(xsp).astype(_BF16),
            "sw1": sw1_l,
            "sw2": sw2_l,
            "sproj": sproj_l,
            "xt_g": _part_layout(xg).astype(_BF16),
            "gate": gate_e,
            "ew1": _w1_layout(ew1[e]).astype(_BF16),
            "ew2": _w1_layout(ew2[e]).astype(_BF16),
            "eproj": _part_layout(eproj[e]).astype(_BF16),
        })

    nc = _build(TS, C, D, H, E)
    res = run_bass_kernel_spmd(
        nc, in_maps, list(range(NCORES)),
        trace=TRACE, trace_cores=list(range(NCORES)) if TRACE else None,
    )
    LAST_RESULTS = res

    out = np.empty((NTOK, D), np.float32)
    probs = np.empty((NTOK, E), np.float32)
    for e in range(NCORES):
        r = res.results[e]
        probs[e * TS:(e + 1) * TS] = r["probs_o"]
        out[e * TS:(e + 1) * TS] = _unpart_layout(r["shared_o"]).T
    for e in range(NCORES):
        te = tok_lists[e]
        yg = _unpart_layout(res.results[e]["yg_o"]).T  # [C, D]
        out[te] += yg[:len(te)]

    return out.reshape(B, T, D), probs.reshape(B, T, E)
